# revision 1
# baseline (speedup 1.0000x reference)
"""NeuralODE forward (Euler, whole-sequence) on 8 Trainium2 NeuronCores.

Math (per step): z <- z + h * (tanh([z, u] @ W1 + b1) @ W2 + b2)
Shapes: z0 [4096, 256], u [4096, 64], W1 [320, 1024], W2 [1024, 256],
t [11]. The fp32 time grid makes ceil(|dt|/0.05) land on 2 for six of
the ten intervals, so the faithful replication of the reference runs
16 Euler micro-steps (matches the reference to 2.7e-7 in fp64).

Sharding: data-parallel over the batch axis. Each core owns 512 rows,
weights replicated, no cross-device communication.

Device layout: state kept transposed (features on partitions, batch on
the free axis) so both matmuls stream batch as the moving free dim:
    hT = tanh(W1z.T @ zT + c_ub)     [1024, 512]
    zT <- zT + (W2h.T @ hT + b2h)    [256, 512]
with c_ub = W1u.T @ uT + b1 hoisted out of the loop (u is constant in
time), W2h = h*W2 / b2h = h*b2 pre-scaled on the host per unique h.
Per step the PE runs 32 K=128 N=512 matmuls (the roofline minimum for
this chunking) plus two K=64 u-matmuls for the last two hidden chunks,
whose tanh reads PSUM directly with the b1 bias — that keeps the
end-of-step serial chain (mm1 -> tanh -> mm2 -> z update) off the DVE.
The other six c_ub adds and the two z updates run on the DVE (6 PSUM
slots for ps1 so the PE is never throttled by the DVE drain), tanh on
ACT in 1024-wide pairs. If b2 is nonzero, b2h enters PSUM via a K=1
matmul against a ones row, keeping the z update a single DVE add.
Matmul tiles are float32r (TF32 full-rate PE path, ~1.7e-3
absmax-relative vs the fp32 reference).
"""

import math
import sys

import numpy as np

sys.path.insert(0, "/opt/trn_rl_repo")

import concourse.mybir as mybir
import concourse.tile as tile
from concourse import bacc
from concourse.bass import ts
from concourse.bass_utils import run_bass_kernel_spmd

H_MAX = 0.05
N_CORES = 8
P = 128

B = 512  # batch rows per core
D = 256  # z dim -> 2 partition chunks
U = 64  # u dim
H = 1024  # hidden -> 8 partition chunks
KD = D // P  # 2
KH = H // P  # 8

MAX_UNIQUE_H = 8
TRACE = False  # set by test.py to collect a HW profile
TRACE_DIR = None  # set by test.py: directory for NTFF artifacts

_program_cache: dict = {}


def _steps_from_t(t_np):
    """Replicate the reference's trace-time step derivation.

    Returns a list of (h, emit) pairs: one entry per Euler micro-step;
    emit=True on the last micro-step of each grid interval (that state
    is recorded into the output sequence).
    """
    steps = []
    for i_t in range(t_np.shape[0] - 1):
        t0f, t1f = float(t_np[i_t]), float(t_np[i_t + 1])
        n_steps = int(math.ceil(abs(t1f - t0f) / H_MAX))
        h = np.float32((t1f - t0f) / n_steps)
        for s in range(n_steps):
            steps.append((float(h), s == n_steps - 1))
    return steps


def _build_program(steps, n_uniq, h_idx, b2_zero):
    f32 = mybir.dt.float32
    f32r = mybir.dt.float32r

    nc = bacc.Bacc(
        "TRN2", target_bir_lowering=False, debug=False, num_devices=N_CORES
    )

    # z0 chunks + b1 packed into one 128-partition tensor, and ut+w1u
    # into one 64-partition tensor: each small DMA costs ~1.5us
    # completion latency serially on the HWDGE ring, so fewer is faster
    zb = nc.dram_tensor("zb", [P, 2 * B + KH], f32r, kind="ExternalInput")
    # uT/W1u zero-padded from 64 to 128 rows: K=64 matmuls issue ~100ns
    # slower than full-array ones; zero weight rows contribute nothing
    uw = nc.dram_tensor("uw", [P, B + H], f32r, kind="ExternalInput")
    w1r = nc.dram_tensor("w1r", [P, KD, H], f32r, kind="ExternalInput")
    # per unique h: W2h k-chunks, uploaded one unique at a time
    w2r = nc.dram_tensor("w2r", [n_uniq, P, KH, D], f32r, kind="ExternalInput")
    b2t = nc.dram_tensor("b2t", [1, n_uniq, D], f32r, kind="ExternalInput")
    onesd = nc.dram_tensor("ones", [1, B], f32r, kind="ExternalInput")
    n_rec = sum(1 for _, e in steps if e)
    out = nc.dram_tensor("out", [n_rec, D, B], f32r, kind="ExternalOutput")

    Tanh = mybir.ActivationFunctionType.Tanh
    add = mybir.AluOpType.add

    with tile.TileContext(nc) as tc:
        with (
            tc.tile_pool(name="const", bufs=1) as const,
            tc.tile_pool(name="zpool", bufs=3) as zpool,
            tc.tile_pool(name="hpool", bufs=2) as hpool,
            tc.tile_pool(name="tmp", bufs=3) as tmp,
            tc.tile_pool(name="psum", bufs=3, space="PSUM") as psum,
        ):
            # DMAs ordered by first use: the c_ub prelude matmuls run
            # first on the PE, so ut/w1u/b1 lead; w1/z0 follow for step 0
            # (w1 split in two so it spreads across DMA queues)
            uw_sb = const.tile([P, B + H], f32r)
            nc.sync.dma_start(out=uw_sb[:], in_=uw[:])
            ut_sb = uw_sb[:, :B]
            w1u_sb = uw_sb[:, B:]
            zb_sb = const.tile([P, 2 * B + KH], f32r)
            nc.sync.dma_start(out=zb_sb[:], in_=zb[:])
            b1_sb = zb_sb[:, 2 * B :].bitcast(f32)
            # step 0 reads z straight from the staging tile
            z_cur = [zb_sb[:, :B], zb_sb[:, B : 2 * B]]
            w1_sb = const.tile([P, KD, H], f32r)
            nc.sync.dma_start(out=w1_sb[:], in_=w1r[:])
            # W2h per unique h, the step-0 copy first
            w2_sb = const.tile([P, n_uniq, KH, D], f32r)
            uniq_order = [h_idx[0]] + [q for q in range(n_uniq) if q != h_idx[0]]
            for q in uniq_order:
                nc.sync.dma_start(out=w2_sb[:, q], in_=w2r[q])
            if not b2_zero:
                b2_sb = const.tile([1, n_uniq, D], f32r)
                nc.sync.dma_start(out=b2_sb[:], in_=b2t[:])
                ones_sb = const.tile([1, B], f32r)
                nc.sync.dma_start(out=ones_sb[:], in_=onesd[:])

            # c_ub[m] = W1u.T @ uT + b1[m], hoisted out of the step loop
            # (only for the DVE-handled chunks m < KH-2)
            cub_sb = const.tile([P, KH - 2, B], f32)
            for m in range(KH - 2):
                ps = psum.tile([P, B], f32, tag="ps1", bufs=6)
                nc.tensor.matmul(
                    ps, w1u_sb[:, ts(m, P)], ut_sb[:], start=True, stop=True
                )
                nc.vector.tensor_scalar(
                    out=cub_sb[:, m, :],
                    in0=ps,
                    scalar1=b1_sb[:, m : m + 1],
                    scalar2=None,
                    op0=add,
                )

            rec = 0
            for i_step, (h_i, emit) in enumerate(steps):
                hsel = h_idx[i_step]
                # mm1: pairs of hidden chunks share one 1024-wide tanh
                # (halves ACT overhead), except the last two chunks stay
                # single so the end-of-step chain mm1(m7) -> add -> tanh
                # -> mm2(k7) is as short as possible.
                h_aps = []
                for mp in range(KH // 2 - 1):
                    # two single DVE adds feed one 1024-wide tanh
                    tadd = tmp.tile([P, 2, B], f32, tag="tadd")
                    for j in range(2):
                        m = 2 * mp + j
                        ps1 = psum.tile([P, B], f32, tag="ps1", bufs=6)
                        nc.tensor.matmul(
                            ps1, w1_sb[:, 0, ts(m, P)], z_cur[0][:],
                            start=True, stop=False,
                        )
                        nc.tensor.matmul(
                            ps1, w1_sb[:, 1, ts(m, P)], z_cur[1][:],
                            start=False, stop=True,
                        )
                        nc.vector.tensor_add(
                            tadd[:, j, :], ps1, cub_sb[:, m, :]
                        )
                    ht = hpool.tile([P, 2, B], f32r, tag=f"h{mp}")
                    nc.scalar.activation(ht, tadd, Tanh)
                    h_aps.extend([ht[:, 0, :], ht[:, 1, :]])
                # last two chunks: u-part on the PE (third K=64 matmul)
                # and tanh straight from PSUM with the b1 bias, so the
                # end-of-step chain skips the DVE entirely:
                # mm1(m7) -> tanh -> mm2(k7)
                for m in (KH - 2, KH - 1):
                    ps1 = psum.tile([P, B], f32, tag="ps1", bufs=6)
                    nc.tensor.matmul(
                        ps1, w1_sb[:, 0, ts(m, P)], z_cur[0][:],
                        start=True, stop=False,
                    )
                    nc.tensor.matmul(
                        ps1, w1_sb[:, 1, ts(m, P)], z_cur[1][:],
                        start=False, stop=False,
                    )
                    nc.tensor.matmul(
                        ps1, w1u_sb[:, ts(m, P)], ut_sb[:],
                        start=False, stop=True,
                    )
                    hts = hpool.tile([P, B], f32r, tag=f"h{m}s")
                    nc.scalar.activation(
                        hts, ps1, Tanh, bias=b1_sb[:, m : m + 1]
                    )
                    h_aps.append(hts[:])

                z_new = []
                for n in range(KD):
                    ps2 = psum.tile([P, B], f32, tag="ps2", bufs=2)
                    first = True
                    if not b2_zero:
                        nc.tensor.matmul(
                            ps2,
                            b2_sb[0:1, hsel, ts(n, P)],
                            ones_sb[0:1, :],
                            start=True,
                            stop=False,
                        )
                        first = False
                    for k in range(KH):
                        nc.tensor.matmul(
                            ps2,
                            w2_sb[:, hsel, k, ts(n, P)],
                            h_aps[k],
                            start=first,
                            stop=(k == KH - 1),
                        )
                        first = False
                    znew = zpool.tile([P, B], f32r, tag=f"z{n}")
                    nc.vector.tensor_add(znew, z_cur[n], ps2)
                    if emit:
                        nc.sync.dma_start(out=out[rec, ts(n, P), :], in_=znew[:])
                    z_new.append(znew)
                if emit:
                    rec += 1
                z_cur = z_new

    nc.compile()
    return nc


def kernel(z0, u, t, W1, b1, W2, b2):
    z0 = np.ascontiguousarray(np.asarray(z0, dtype=np.float32))
    u = np.ascontiguousarray(np.asarray(u, dtype=np.float32))
    t_np = np.asarray(t, dtype=np.float32)
    W1 = np.ascontiguousarray(np.asarray(W1, dtype=np.float32))
    b1 = np.ascontiguousarray(np.asarray(b1, dtype=np.float32))
    W2 = np.ascontiguousarray(np.asarray(W2, dtype=np.float32))
    b2 = np.ascontiguousarray(np.asarray(b2, dtype=np.float32))

    bs, dim = z0.shape
    assert (bs, dim) == (N_CORES * B, D), (bs, dim)
    assert u.shape == (bs, U) and W1.shape == (D + U, H)
    assert W2.shape == (H, D) and b1.shape == (H,) and b2.shape == (D,)

    steps = _steps_from_t(t_np)
    n_rec = sum(1 for _, e in steps if e)
    if n_rec == 0:
        return z0[None].copy()

    uniq_h = sorted(set(h for h, _ in steps))
    assert len(uniq_h) <= MAX_UNIQUE_H, (
        f"{len(uniq_h)} unique step sizes; raise MAX_UNIQUE_H"
    )
    h_idx = [uniq_h.index(h) for h, _ in steps]
    n_uniq = len(uniq_h)
    b2_zero = bool(np.all(b2 == 0.0))

    key = (tuple(steps), n_uniq, tuple(h_idx), b2_zero)
    nc = _program_cache.get(key)
    if nc is None:
        nc = _build_program(steps, n_uniq, h_idx, b2_zero)
        _program_cache[key] = nc

    w1r = np.ascontiguousarray(W1[:D].reshape(KD, P, H).transpose(1, 0, 2))
    w1u = np.ascontiguousarray(W1[D:])
    w2k = W2.reshape(KH, P, D).transpose(1, 0, 2)  # [P, KH, D]
    w2r = np.ascontiguousarray(
        np.stack([w2k * np.float32(h) for h in uniq_h], axis=0)
    )  # [n_uniq, P, KH, D]
    b1t = np.ascontiguousarray(b1.reshape(KH, P).T)
    b2t = np.ascontiguousarray(
        np.stack([b2 * np.float32(h) for h in uniq_h], axis=0)[None]
    )  # [1, n_uniq, D]

    in_maps = []
    for c in range(N_CORES):
        sl = slice(c * B, (c + 1) * B)
        z0cT = z0[sl].T  # [D, B]
        in_maps.append(
            {
                "zb": np.ascontiguousarray(
                    np.concatenate([z0cT[:P], z0cT[P:], b1t], axis=1)
                ),
                "uw": np.ascontiguousarray(
                    np.concatenate(
                        [
                            np.concatenate([u[sl].T, w1u], axis=1),
                            np.zeros((P - U, B + H), np.float32),
                        ],
                        axis=0,
                    )
                ),
                "w1r": w1r,
                "w2r": w2r,
                "b2t": b2t,
                "ones": np.ones((1, B), dtype=np.float32),
            }
        )

    res = run_bass_kernel_spmd(
        nc, in_maps, list(range(N_CORES)), trace=TRACE, tmpdir=TRACE_DIR
    )
    kernel.last_results = res

    full = np.empty((n_rec + 1, bs, dim), dtype=np.float32)
    full[0] = z0
    for c in range(N_CORES):
        o = res.results[c]["out"]  # [n_rec, D, B] transposed states
        full[1:, c * B : (c + 1) * B, :] = o.transpose(0, 2, 1)
    return full

